# revision 4
# baseline (speedup 1.0000x reference)
"""BitNet ternary layer on 8 trn2 NeuronCores — v2.

y[b,s,o] = sum_i x[b,s,i] * tq(w)[o,i],  tq(w) = sign(w) * (|w| > 0.7*mean|w|)

Distribution: data-parallel over the batch dim — core c gets x[c] and a
replicated copy of the weight. Host-side marshaling only: x is pre-cast to
bf16 and pre-transposed to xT [I, S]; w is pre-transposed to wT [I, O]
(both are layout/dtype repacks — all of the layer's math stays on device).

Per core:
  A) absmean pass: stream wT fp32 [128, O] i-tiles, DVE abs-reduce to
     per-partition partials, collapse on the PE, scale by 0.7/(O*I) -> t.
  B) for each S-half (SH=1024) and each o-chunk (OC=512):
     quantize: stream wT fp32 [128, KG*OC] tiles, two DVE compares
     (w > t) - (w < -t) -> ternary bf16 directly in matmul-ready
     wqT [i-part, o-free] layout (no transposes anywhere);
     matmul: 8 s-blocks x 32 k-tiles of N=512 bf16 matmuls accumulating
     into PSUM; ACT drains PSUM -> SBUF fp32; DMA stores y [s, o].

Engine budget per core: PE 4096 matmuls ~875us (bottleneck), DMA ~240MB
~670us, DVE ~420us, ACT ~350us. The absmean pass (~180us) serializes at
the start (t gates quantization).
"""
import copy
import sys

sys.path.insert(0, '/opt/trn_rl_repo')

import numpy as np
import ml_dtypes

import concourse.bass as bass
from concourse import mybir
from concourse.tile import TileContext
from concourse.vector_clock import ScopedClock
from concourse.bass_utils import run_bass_kernel_spmd

# ---------------------------------------------------------------------------
# Workarounds for this container's walrus build rejecting sem-waits attached
# to several instruction structs (CTRL/NoOp/Drain/DMA-transpose): emit the
# TileContext exit waits as standalone wait_ge instructions, and post-process
# the module to hoist every immediate sem-wait onto its own single-wait
# InstEventSemaphore (same engine, same program position -> same semantics).
# ---------------------------------------------------------------------------


def _patched_drain_and_barrier(self, tick_clock, wait_clock):
    probe = self.nc.sync.nop()
    wait_clock.add_sem_waits(probe.ins, ScopedClock({None: tick_clock.global_clock}))
    si = probe.ins.sync_info
    waits = list(si.on_wait) if si is not None else []
    if waits:
        probe.ins.sync_info = mybir.SyncInfo(on_wait=[], on_update=list(si.on_update))
        for w in waits:
            self.nc.sync.wait_ge(bass.SemaphoreHandle(w.ant_name, w.id), w.wait_value)
    self.nc.sync.drain()
    self.nc.all_engine_barrier()
    assert self.sems is not None
    popped = self.nc._tile_sem_poison_stack.pop()
    assert popped is self._sem_poison
    self.nc.clear_and_free_semaphores(list(self.sems.allocated().values()))
    self.nc.all_engine_barrier()


TileContext._drain_and_barrier = _patched_drain_and_barrier

_ctr = [0]


def _hoist_waits(nc):
    new_module = copy.replace(nc.m, functions=[])
    for function in nc.m.functions:
        new_function = copy.replace(function, blocks=[])
        new_function.set_allocations_from_list(function.allocations)
        for block in function.blocks:
            new_insts = []
            for inst in block.instructions:
                si = inst.sync_info
                if si is not None and not isinstance(inst, mybir.InstEventSemaphore):
                    imm = [w for w in si.on_wait if w.wait_reg is None]
                    if imm:
                        reg = [w for w in si.on_wait if w.wait_reg is not None]
                        for w in imm:
                            _ctr[0] += 1
                            ev = mybir.InstEventSemaphore(
                                name=f"HW-{_ctr[0]}", ins=[], outs=[])
                            ev.engine = inst.engine
                            ev.sync_info = mybir.SyncInfo(on_wait=[w], on_update=[])
                            new_insts.append(ev)
                        inst.sync_info = mybir.SyncInfo(
                            on_wait=reg, on_update=list(si.on_update))
                new_insts.append(inst)
            new_block = copy.replace(block, instructions=new_insts)
            new_function.blocks.append(new_block)
        new_module.functions.append(new_function)
    nc.m = new_module
    return nc


# ---------------------------------------------------------------------------
# Problem shapes (hardcoded per spec)
# ---------------------------------------------------------------------------
B = 8            # batch -> one per core
S = 2048         # tokens per core
I = 4096         # in features (contraction)
O = 4096         # out features
P = 128
NK = I // P      # 32 k-tiles
OC = 512         # o-chunk width (one PSUM bank at fp32)
NOC = O // OC    # 8
SH = 1024        # token half kept SBUF-resident as xT
NH = S // SH     # 2 halves
NSB = SH // P    # 8 s-tiles per half
KG = 4           # k-tiles per quantize load ([128, KG, OC] fp32 = 1MB DMA)
NKG = NK // KG   # 8 quantize loads per o-chunk


def build_program(reps=1):
    fp32 = mybir.dt.float32
    bf16 = mybir.dt.bfloat16

    nc = bass.Bass()
    # xT: x[c] transposed to [I, S], bf16.  wT: weight transposed to [I, O].
    xT_in = nc.declare_dram_parameter("xT", [I, S], bf16, isOutput=False)
    wT_in = nc.declare_dram_parameter("wT", [I, O], fp32, isOutput=False)
    y_out = nc.declare_dram_parameter("y", [S, O], fp32, isOutput=True)

    with TileContext(nc) as tc:
        with (
            tc.tile_pool(name="dram", bufs=1, space="DRAM") as dram,
            tc.tile_pool(name="singles", bufs=1) as singles,
            tc.tile_pool(name="psum1", bufs=1, space="PSUM") as psum1,
            tc.tile_pool(name="psum", bufs=6, space="PSUM") as psum_pool,
            tc.tile_pool(name="outsb", bufs=4) as outsb,
            tc.tile_pool(name="xh_pool", bufs=1) as xh_pool,
            tc.tile_pool(name="wqT_pool", bufs=2) as wqT_pool,
            tc.tile_pool(name="win_pool", bufs=3) as win_pool,
            tc.tile_pool(name="qtmp", bufs=2) as qtmp,
        ):
            t_dram = dram.tile([1, 1], fp32)
            NA = NK * 2  # absmean tiles: [P, O//2] fp32, shares win slots
            partials = singles.tile([P, NA], fp32)
            part1 = singles.tile([P, 1], fp32)
            ones = singles.tile([P, 1], fp32)
            tval = singles.tile([1, 1], fp32)
            t_b = singles.tile([P, 1], fp32)
            nt_b = singles.tile([P, 1], fp32)

            for rep in range(reps):
                # ---- Phase A: threshold t = 0.7 * mean|w| (exact fp32) ----
                for j in range(NA):
                    wa = win_pool.tile([P, KG * OC], fp32, tag="win")
                    nc.sync.dma_start(
                        out=wa[:],
                        in_=wT_in[(j // 2) * P:(j // 2 + 1) * P,
                                  (j % 2) * (O // 2):(j % 2 + 1) * (O // 2)])
                    nc.vector.tensor_reduce(
                        partials[:, j:j + 1], wa[:],
                        axis=mybir.AxisListType.X,
                        op=mybir.AluOpType.add,
                        apply_absolute_value=True)
                nc.vector.tensor_reduce(
                    part1[:], partials[:], axis=mybir.AxisListType.X,
                    op=mybir.AluOpType.add)
                nc.vector.memset(ones[:], 1.0)
                tsum = psum1.tile([1, 1], fp32)
                nc.tensor.matmul(tsum[:], lhsT=part1[:], rhs=ones[:],
                                 start=True, stop=True)
                nc.scalar.activation(tval[:], tsum[:],
                                     mybir.ActivationFunctionType.Copy,
                                     scale=0.7 / float(O * I))
                nc.sync.dma_start(out=t_dram[:], in_=tval[:])
                t_bcast_ap = bass.AP(
                    tensor=t_dram.tensor, offset=t_dram.offset,
                    ap=[[0, P], [1, 1]])
                nc.gpsimd.dma_start(out=t_b[:], in_=t_bcast_ap)
                nc.vector.tensor_scalar_mul(nt_b[:], t_b[:], -1.0)

                # ---- Phase B: per-half quantize + matmul pipeline ----
                for h in range(NH):
                    # xT half: [128, NK, SH] bf16 (64KB/partition), one DMA
                    xh = xh_pool.tile([P, NK, SH], bf16)
                    xh_src = bass.AP(
                        tensor=xT_in, offset=h * SH,
                        ap=[[S, P], [P * S, NK], [1, SH]])
                    nc.sync.dma_start(out=xh[:], in_=xh_src)
                    for oc in range(NOC):
                        # quantize chunk oc: wqT [128, NK, OC] ternary bf16
                        wqT = wqT_pool.tile([P, NK, OC], bf16)
                        for g in range(NKG):
                            k0 = g * KG
                            win = win_pool.tile([P, KG, OC], fp32)
                            w_src = bass.AP(
                                tensor=wT_in,
                                offset=k0 * P * O + oc * OC,
                                ap=[[O, P], [P * O, KG], [1, OC]])
                            nc.sync.dma_start(out=win[:], in_=w_src)
                            pt = qtmp.tile([P, KG, OC], bf16, tag="pt")
                            nt = qtmp.tile([P, KG, OC], bf16, tag="nt")
                            nc.vector.tensor_scalar(
                                pt[:], win[:], t_b[:], None,
                                op0=mybir.AluOpType.is_gt)
                            nc.vector.tensor_scalar(
                                nt[:], win[:], nt_b[:], None,
                                op0=mybir.AluOpType.is_lt)
                            nc.vector.tensor_sub(
                                wqT[:, k0:k0 + KG, :], pt[:], nt[:])
                        # matmul: 8 s-blocks x 32 k, N=512
                        for s in range(NSB):
                            ps = psum_pool.tile([P, OC], fp32)
                            for k in range(NK):
                                nc.tensor.matmul(
                                    ps[:],
                                    lhsT=xh[:, k, s * P:(s + 1) * P],
                                    rhs=wqT[:, k, :],
                                    start=(k == 0),
                                    stop=(k == NK - 1))
                            ob = outsb.tile([P, OC], fp32)
                            nc.scalar.activation(
                                ob[:], ps[:],
                                mybir.ActivationFunctionType.Copy)
                            nc.scalar.dma_start(
                                out=y_out[
                                    h * SH + s * P:h * SH + (s + 1) * P,
                                    oc * OC:(oc + 1) * OC],
                                in_=ob[:])
                if reps > 1:
                    tc.strict_bb_all_engine_barrier()

    _hoist_waits(nc)
    return nc


_program_cache = {}


def _get_program(reps=1):
    if reps not in _program_cache:
        _program_cache[reps] = build_program(reps=reps)
    return _program_cache[reps]


def make_inputs(x, weight):
    """Host-side marshaling: slice/cast/transpose into per-core input maps."""
    x = np.asarray(x, dtype=np.float32)
    weight = np.asarray(weight, dtype=np.float32)
    assert x.shape == (B, S, I), x.shape
    assert weight.shape == (O, I), weight.shape
    wT = np.ascontiguousarray(weight.T)
    in_maps = []
    for c in range(B):
        xT = np.ascontiguousarray(x[c].T).astype(ml_dtypes.bfloat16)
        in_maps.append({"xT": xT, "wT": wT})
    return in_maps


def run(x, weight, trace=False, reps=1):
    nc = _get_program(reps=reps)
    in_maps = make_inputs(x, weight)
    res = run_bass_kernel_spmd(nc, in_maps, list(range(B)), trace=trace)
    y = np.stack([res.results[c]["y"] for c in range(B)], axis=0)
    return y.astype(np.float32), res


def kernel(x, weight):
    y, _ = run(x, weight)
    return y


# revision 18
# speedup vs baseline: 74.4744x; 74.4744x over previous
"""BitNet ternary layer on 8 trn2 NeuronCores — v2.

y[b,s,o] = sum_i x[b,s,i] * tq(w)[o,i],  tq(w) = sign(w) * (|w| > 0.7*mean|w|)

Distribution: data-parallel over the batch dim — core c gets x[c] and a
replicated copy of the weight. Host-side marshaling only: x is pre-cast to
bf16 and pre-transposed to xT [I, S]; w is pre-transposed to wT [I, O]
(both are layout/dtype repacks — all of the layer's math stays on device).

Per core:
  A) absmean pass: stream wT fp32 [128, O] i-tiles, DVE abs-reduce to
     per-partition partials, collapse on the PE, scale by 0.7/(O*I) -> t.
  B) for each S-half (SH=1024) and each o-chunk (OC=512):
     quantize: stream wT fp32 [128, KG*OC] tiles, two DVE compares
     (w > t) - (w < -t) -> ternary bf16 directly in matmul-ready
     wqT [i-part, o-free] layout (no transposes anywhere);
     matmul: 8 s-blocks x 32 k-tiles of N=512 bf16 matmuls accumulating
     into PSUM; ACT drains PSUM -> SBUF fp32; DMA stores y [s, o].

Engine budget per core: PE 4096 matmuls ~875us (bottleneck), DMA ~240MB
~670us, DVE ~420us, ACT ~350us. The absmean pass (~180us) serializes at
the start (t gates quantization).
"""
import copy
import sys

sys.path.insert(0, '/opt/trn_rl_repo')

import numpy as np
import ml_dtypes

import concourse.bass as bass
from concourse import mybir
from concourse.tile import TileContext
from concourse.vector_clock import ScopedClock
from concourse.bass_utils import run_bass_kernel_spmd

# ---------------------------------------------------------------------------
# Workarounds for this container's walrus build rejecting sem-waits attached
# to several instruction structs (CTRL/NoOp/Drain/DMA-transpose): emit the
# TileContext exit waits as standalone wait_ge instructions, and post-process
# the module to hoist every immediate sem-wait onto its own single-wait
# InstEventSemaphore (same engine, same program position -> same semantics).
# ---------------------------------------------------------------------------


def _patched_drain_and_barrier(self, tick_clock, wait_clock):
    probe = self.nc.sync.nop()
    wait_clock.add_sem_waits(probe.ins, ScopedClock({None: tick_clock.global_clock}))
    si = probe.ins.sync_info
    waits = list(si.on_wait) if si is not None else []
    if waits:
        probe.ins.sync_info = mybir.SyncInfo(on_wait=[], on_update=list(si.on_update))
        for w in waits:
            self.nc.sync.wait_ge(bass.SemaphoreHandle(w.ant_name, w.id), w.wait_value)
    self.nc.sync.drain()
    self.nc.all_engine_barrier()
    assert self.sems is not None
    popped = self.nc._tile_sem_poison_stack.pop()
    assert popped is self._sem_poison
    self.nc.clear_and_free_semaphores(list(self.sems.allocated().values()))
    self.nc.all_engine_barrier()


TileContext._drain_and_barrier = _patched_drain_and_barrier

_ctr = [0]


def _hoist_waits(nc):
    new_module = copy.replace(nc.m, functions=[])
    for function in nc.m.functions:
        new_function = copy.replace(function, blocks=[])
        new_function.set_allocations_from_list(function.allocations)
        for block in function.blocks:
            new_insts = []
            for inst in block.instructions:
                si = inst.sync_info
                if si is not None and not isinstance(inst, mybir.InstEventSemaphore):
                    imm = [w for w in si.on_wait if w.wait_reg is None]
                    if imm:
                        reg = [w for w in si.on_wait if w.wait_reg is not None]
                        for w in imm:
                            _ctr[0] += 1
                            ev = mybir.InstEventSemaphore(
                                name=f"HW-{_ctr[0]}", ins=[], outs=[])
                            ev.engine = inst.engine
                            ev.sync_info = mybir.SyncInfo(on_wait=[w], on_update=[])
                            new_insts.append(ev)
                        inst.sync_info = mybir.SyncInfo(
                            on_wait=reg, on_update=list(si.on_update))
                new_insts.append(inst)
            new_block = copy.replace(block, instructions=new_insts)
            new_function.blocks.append(new_block)
        new_module.functions.append(new_function)
    nc.m = new_module
    return nc


# ---------------------------------------------------------------------------
# Problem shapes (hardcoded per spec)
# ---------------------------------------------------------------------------
B = 8            # batch -> one per core
S = 2048         # tokens per core
I = 4096         # in features (contraction)
O = 4096         # out features
P = 128
NK = I // P      # 32 k-tiles
OC = 512         # o-chunk width (one PSUM bank at fp32)
NOC = O // OC    # 8
SH = 1024        # token half kept SBUF-resident as xT
NH = S // SH     # 2 halves
NSB = SH // P    # 8 s-tiles per half
KG = 4           # k-tiles per quantize load ([128, KG, OC] fp32 = 1MB DMA)
NKG = NK // KG   # 8 quantize loads per o-chunk
IP = I // B      # 512: per-core slice of i-rows for the sharded absmean
XQ = 4           # xh split into XQ s-quarter tiles for finer WAR deps


def build_program(reps=1, collective=True):
    fp32 = mybir.dt.float32
    bf16 = mybir.dt.bfloat16

    nc = bass.Bass(num_devices=B if collective else None)
    # xT: x[c] transposed to [I, S], bf16.  wT: weight transposed to [I, O].
    # wp: this core's [IP, O] slice of wT (absmean sharding; host-sliced).
    xT_in = nc.declare_dram_parameter("xT", [I, S], bf16, isOutput=False)
    wT_in = nc.declare_dram_parameter("wT", [I, O], fp32, isOutput=False)
    wp_in = nc.declare_dram_parameter("wp", [IP, O], fp32, isOutput=False)
    y_out = nc.declare_dram_parameter("y", [S, O], fp32, isOutput=True)
    cc_in = nc.dram_tensor("cc_in", [P, 1], fp32)
    cc_out = nc.dram_tensor("cc_out", [P, 1], fp32,
                            addr_space="Shared" if collective else "Local")

    with TileContext(nc) as tc:
        with (
            tc.tile_pool(name="dram", bufs=1, space="DRAM") as dram,
            tc.tile_pool(name="singles", bufs=1) as singles,
            tc.tile_pool(name="psum1", bufs=1, space="PSUM") as psum1,
            tc.tile_pool(name="psum", bufs=6, space="PSUM") as psum_pool,
            tc.tile_pool(name="outsb", bufs=4) as outsb,
            tc.tile_pool(name="xh_pool", bufs=4) as xh_pool,
            tc.tile_pool(name="wqT_pool", bufs=2 * NKG) as wqT_pool,
            tc.tile_pool(name="win_pool", bufs=3) as win_pool,
            tc.tile_pool(name="qtmp", bufs=2) as qtmp,
        ):
            NA = (IP // P) * 2  # absmean tiles: [P, O//2] fp32 over wp
            partials = singles.tile([P, NA], fp32)
            part1 = singles.tile([P, 1], fp32)
            parts_g = singles.tile([P, 1], fp32)
            ones = singles.tile([P, 1], fp32)
            ones_row = singles.tile([1, P], fp32)
            tval = singles.tile([1, 1], fp32)
            t_b = singles.tile([P, 1], fp32)
            nt_b = singles.tile([P, 1], fp32)

            for rep in range(reps):
                # ---- Phase A: threshold t = 0.7 * mean|w| (exact fp32).
                # Each core abs-sums its own [IP, O] slice; partials are
                # AllReduced across the 8 cores. Dedicated pool so phase-B
                # prefetches don't WAR-serialize behind absmean slots.
                with tc.tile_pool(name="pha", bufs=2) as pha:
                    for j in range(NA):
                        wa = pha.tile([P, KG * OC], fp32)
                        nc.sync.dma_start(
                            out=wa[:],
                            in_=wp_in[(j // 2) * P:(j // 2 + 1) * P,
                                      (j % 2) * (O // 2):(j % 2 + 1) * (O // 2)])
                        nc.vector.tensor_reduce(
                            partials[:, j:j + 1], wa[:],
                            axis=mybir.AxisListType.X,
                            op=mybir.AluOpType.add,
                            apply_absolute_value=True)
                nc.vector.tensor_reduce(
                    part1[:], partials[:], axis=mybir.AxisListType.X,
                    op=mybir.AluOpType.add)
                nc.gpsimd.dma_start(out=cc_in[:], in_=part1[:])
                if collective:
                    nc.gpsimd.collective_compute(
                        "AllReduce", mybir.AluOpType.add,
                        replica_groups=[list(range(B))],
                        ins=[cc_in[:]], outs=[cc_out[:]])
                else:
                    nc.gpsimd.dma_start(out=cc_out[:], in_=cc_in[:])
                nc.gpsimd.dma_start(out=parts_g[:], in_=cc_out[:])
                nc.vector.memset(ones[:], 1.0)
                nc.vector.memset(ones_row[:], 1.0)
                tsum = psum1.tile([1, 1], fp32)
                nc.tensor.matmul(tsum[:], lhsT=parts_g[:], rhs=ones[:],
                                 start=True, stop=True)
                nc.scalar.activation(tval[:], tsum[:],
                                     mybir.ActivationFunctionType.Copy,
                                     scale=0.7 / float(O * I))
                # broadcast t across partitions on the PE (no DRAM roundtrip)
                tb_ps = psum1.tile([P, 1], fp32, tag="tbps")
                nc.tensor.matmul(tb_ps[:], lhsT=ones_row[:], rhs=tval[:],
                                 start=True, stop=True)
                nc.scalar.activation(t_b[:], tb_ps[:],
                                     mybir.ActivationFunctionType.Copy)
                nc.vector.tensor_scalar_mul(nt_b[:], t_b[:], -1.0)

                # ---- Phase B: per-half quantize + matmul pipeline ----
                preload = {}
                for h in range(NH):
                    if h == 0:
                        # prefetch the first win groups of oc=0 ahead of the
                        # bulky xh loads so the first quantize isn't queued
                        # behind 8MB of x on the same DMA ring
                        for g in range(3):
                            win = win_pool.tile([P, KG, OC], fp32, tag="win")
                            w_src = bass.AP(
                                tensor=wT_in, offset=g * KG * P * O,
                                ap=[[O, P], [P * O, KG], [1, OC]])
                            nc.sync.dma_start(out=win[:], in_=w_src)
                            preload[g] = win
                    # xT half as XQ s-quarter tiles (finer WAR granularity so
                    # h=1 quarters can load under h=0's tail matmuls). For
                    # h=0 only q0 is loaded up front; q1-3 are emitted after
                    # the first quantize chunk to keep the DMA queue clear
                    # for the threshold chain.
                    xq = SH // XQ

                    def _load_xh(q, h=h, xq=xq):
                        xh_t = xh_pool.tile([P, NK, xq], bf16, tag="xh")
                        xh_src = bass.AP(
                            tensor=xT_in, offset=h * SH + q * xq,
                            ap=[[S, P], [P * S, NK], [1, xq]])
                        nc.sync.dma_start(out=xh_t[:], in_=xh_src)
                        return xh_t

                    xhq = [_load_xh(0)]
                    if h > 0:
                        xhq += [_load_xh(q) for q in range(1, XQ)]
                    for oc in range(NOC):
                        # quantize chunk oc into NKG per-k-group sub-tiles
                        # [128, KG, OC] ternary bf16 (fine deps: k=0 matmuls
                        # start as soon as the first group is quantized)
                        wq_g = []
                        for g in range(NKG):
                            k0 = g * KG
                            if h == 0 and oc == 0 and g in preload:
                                win = preload.pop(g)
                            else:
                                win = win_pool.tile([P, KG, OC], fp32,
                                                    tag="win")
                                w_src = bass.AP(
                                    tensor=wT_in,
                                    offset=k0 * P * O + oc * OC,
                                    ap=[[O, P], [P * O, KG], [1, OC]])
                                nc.sync.dma_start(out=win[:], in_=w_src)
                            pt = qtmp.tile([P, KG, OC], bf16, tag="pt")
                            nt = qtmp.tile([P, KG, OC], bf16, tag="nt")
                            wq = wqT_pool.tile([P, KG, OC], bf16)
                            nc.vector.tensor_scalar(
                                pt[:], win[:], t_b[:], None,
                                op0=mybir.AluOpType.is_gt)
                            nc.vector.tensor_scalar(
                                nt[:], win[:], nt_b[:], None,
                                op0=mybir.AluOpType.is_lt)
                            nc.vector.tensor_sub(wq[:], pt[:], nt[:])
                            wq_g.append(wq)
                        if h == 0 and oc == 0:
                            xhq += [_load_xh(q) for q in range(1, XQ)]
                        # matmul: 8 s-blocks x 32 k, N=512
                        for s in range(NSB):
                            sq, so = divmod(s, NSB // XQ)
                            ps = psum_pool.tile([P, OC], fp32)
                            for k in range(NK):
                                nc.tensor.matmul(
                                    ps[:],
                                    lhsT=xhq[sq][:, k, so * P:(so + 1) * P],
                                    rhs=wq_g[k // KG][:, k % KG, :],
                                    start=(k == 0),
                                    stop=(k == NK - 1))
                            ob = outsb.tile([P, OC], fp32)
                            nc.scalar.activation(
                                ob[:], ps[:],
                                mybir.ActivationFunctionType.Copy)
                            nc.scalar.dma_start(
                                out=y_out[
                                    h * SH + s * P:h * SH + (s + 1) * P,
                                    oc * OC:(oc + 1) * OC],
                                in_=ob[:])
                if reps > 1:
                    tc.strict_bb_all_engine_barrier()

    _hoist_waits(nc)
    return nc


_program_cache = {}


def _get_program(reps=1):
    if reps not in _program_cache:
        _program_cache[reps] = build_program(reps=reps)
    return _program_cache[reps]


def make_inputs(x, weight):
    """Host-side marshaling: slice/cast/transpose into per-core input maps."""
    x = np.asarray(x, dtype=np.float32)
    weight = np.asarray(weight, dtype=np.float32)
    assert x.shape == (B, S, I), x.shape
    assert weight.shape == (O, I), weight.shape
    wT = np.ascontiguousarray(weight.T)
    in_maps = []
    for c in range(B):
        xT = np.ascontiguousarray(x[c].T).astype(ml_dtypes.bfloat16)
        wp = np.ascontiguousarray(wT[c * IP:(c + 1) * IP, :])
        in_maps.append({"xT": xT, "wT": wT, "wp": wp})
    return in_maps


def run(x, weight, trace=False, reps=1):
    nc = _get_program(reps=reps)
    in_maps = make_inputs(x, weight)
    res = run_bass_kernel_spmd(nc, in_maps, list(range(B)), trace=trace)
    y = np.stack([res.results[c]["y"] for c in range(B)], axis=0)
    return y.astype(np.float32), res


def kernel(x, weight):
    y, _ = run(x, weight)
    return y


# revision 21
# speedup vs baseline: 75.3044x; 1.0111x over previous
"""BitNet ternary layer on 8 trn2 NeuronCores — v2.

y[b,s,o] = sum_i x[b,s,i] * tq(w)[o,i],  tq(w) = sign(w) * (|w| > 0.7*mean|w|)

Distribution: data-parallel over the batch dim — core c gets x[c] and a
replicated copy of the weight. Host-side marshaling only: x is pre-cast to
bf16 and pre-transposed to xT [I, S]; w is pre-transposed to wT [I, O]
(both are layout/dtype repacks — all of the layer's math stays on device).

Per core:
  A) absmean pass: stream wT fp32 [128, O] i-tiles, DVE abs-reduce to
     per-partition partials, collapse on the PE, scale by 0.7/(O*I) -> t.
  B) for each S-half (SH=1024) and each o-chunk (OC=512):
     quantize: stream wT fp32 [128, KG*OC] tiles, two DVE compares
     (w > t) - (w < -t) -> ternary bf16 directly in matmul-ready
     wqT [i-part, o-free] layout (no transposes anywhere);
     matmul: 8 s-blocks x 32 k-tiles of N=512 bf16 matmuls accumulating
     into PSUM; ACT drains PSUM -> SBUF fp32; DMA stores y [s, o].

Engine budget per core: PE 4096 matmuls ~875us (bottleneck), DMA ~240MB
~670us, DVE ~420us, ACT ~350us. The absmean pass (~180us) serializes at
the start (t gates quantization).
"""
import copy
import sys

sys.path.insert(0, '/opt/trn_rl_repo')

import numpy as np
import ml_dtypes

import concourse.bass as bass
from concourse import mybir
from concourse.tile import TileContext
from concourse.vector_clock import ScopedClock
from concourse.bass_utils import run_bass_kernel_spmd

# ---------------------------------------------------------------------------
# Workarounds for this container's walrus build rejecting sem-waits attached
# to several instruction structs (CTRL/NoOp/Drain/DMA-transpose): emit the
# TileContext exit waits as standalone wait_ge instructions, and post-process
# the module to hoist every immediate sem-wait onto its own single-wait
# InstEventSemaphore (same engine, same program position -> same semantics).
# ---------------------------------------------------------------------------


def _patched_drain_and_barrier(self, tick_clock, wait_clock):
    probe = self.nc.sync.nop()
    wait_clock.add_sem_waits(probe.ins, ScopedClock({None: tick_clock.global_clock}))
    si = probe.ins.sync_info
    waits = list(si.on_wait) if si is not None else []
    if waits:
        probe.ins.sync_info = mybir.SyncInfo(on_wait=[], on_update=list(si.on_update))
        for w in waits:
            self.nc.sync.wait_ge(bass.SemaphoreHandle(w.ant_name, w.id), w.wait_value)
    self.nc.sync.drain()
    self.nc.all_engine_barrier()
    assert self.sems is not None
    popped = self.nc._tile_sem_poison_stack.pop()
    assert popped is self._sem_poison
    self.nc.clear_and_free_semaphores(list(self.sems.allocated().values()))
    self.nc.all_engine_barrier()


TileContext._drain_and_barrier = _patched_drain_and_barrier

_ctr = [0]


def _hoist_waits(nc):
    new_module = copy.replace(nc.m, functions=[])
    for function in nc.m.functions:
        new_function = copy.replace(function, blocks=[])
        new_function.set_allocations_from_list(function.allocations)
        for block in function.blocks:
            new_insts = []
            for inst in block.instructions:
                si = inst.sync_info
                if si is not None and not isinstance(inst, mybir.InstEventSemaphore):
                    imm = [w for w in si.on_wait if w.wait_reg is None]
                    if imm:
                        reg = [w for w in si.on_wait if w.wait_reg is not None]
                        for w in imm:
                            _ctr[0] += 1
                            ev = mybir.InstEventSemaphore(
                                name=f"HW-{_ctr[0]}", ins=[], outs=[])
                            ev.engine = inst.engine
                            ev.sync_info = mybir.SyncInfo(on_wait=[w], on_update=[])
                            new_insts.append(ev)
                        inst.sync_info = mybir.SyncInfo(
                            on_wait=reg, on_update=list(si.on_update))
                new_insts.append(inst)
            new_block = copy.replace(block, instructions=new_insts)
            new_function.blocks.append(new_block)
        new_module.functions.append(new_function)
    nc.m = new_module
    return nc


# ---------------------------------------------------------------------------
# Problem shapes (hardcoded per spec)
# ---------------------------------------------------------------------------
B = 8            # batch -> one per core
S = 2048         # tokens per core
I = 4096         # in features (contraction)
O = 4096         # out features
P = 128
NK = I // P      # 32 k-tiles
OC = 512         # o-chunk width (one PSUM bank at fp32)
NOC = O // OC    # 8
SH = 1024        # token half kept SBUF-resident as xT
NH = S // SH     # 2 halves
NSB = SH // P    # 8 s-tiles per half
KG = 4           # k-tiles per quantize load ([128, KG, OC] fp32 = 1MB DMA)
NKG = NK // KG   # 8 quantize loads per o-chunk
IP = I // B      # 512: per-core slice of i-rows for the sharded absmean
XQ = 4           # xh split into XQ s-quarter tiles for finer WAR deps


def build_program(reps=1, collective=True):
    fp32 = mybir.dt.float32
    bf16 = mybir.dt.bfloat16

    nc = bass.Bass(num_devices=B if collective else None)
    # xT: x[c] transposed to [I, S], bf16.  wT: weight transposed to [I, O].
    # wp: this core's [IP, O] slice of wT (absmean sharding; host-sliced).
    # y is written bf16 (host casts back to fp32; ~1e-3 extra rel err).
    xT_in = nc.declare_dram_parameter("xT", [I, S], bf16, isOutput=False)
    wT_in = nc.declare_dram_parameter("wT", [I, O], fp32, isOutput=False)
    wp_in = nc.declare_dram_parameter("wp", [IP, O], fp32, isOutput=False)
    y_out = nc.declare_dram_parameter("y", [S, O], bf16, isOutput=True)
    cc_in = nc.dram_tensor("cc_in", [P, 1], fp32)
    cc_out = nc.dram_tensor("cc_out", [P, 1], fp32,
                            addr_space="Shared" if collective else "Local")

    with TileContext(nc) as tc:
        with (
            tc.tile_pool(name="dram", bufs=1, space="DRAM") as dram,
            tc.tile_pool(name="singles", bufs=1) as singles,
            tc.tile_pool(name="psum1", bufs=1, space="PSUM") as psum1,
            tc.tile_pool(name="psum", bufs=6, space="PSUM") as psum_pool,
            tc.tile_pool(name="outsb", bufs=4) as outsb,
            tc.tile_pool(name="xh_pool", bufs=4) as xh_pool,
            tc.tile_pool(name="wqT_pool", bufs=2 * NKG) as wqT_pool,
            tc.tile_pool(name="win_pool", bufs=4) as win_pool,
            tc.tile_pool(name="qtmp", bufs=2) as qtmp,
        ):
            NA = (IP // P) * 2  # absmean tiles: [P, O//2] fp32 over wp
            partials = singles.tile([P, NA], fp32)
            part1 = singles.tile([P, 1], fp32)
            parts_g = singles.tile([P, 1], fp32)
            ones = singles.tile([P, 1], fp32)
            ones_row = singles.tile([1, P], fp32)
            tval = singles.tile([1, 1], fp32)
            t_b = singles.tile([P, 1], fp32)
            nt_b = singles.tile([P, 1], fp32)

            for rep in range(reps):
                # ---- Phase A: threshold t = 0.7 * mean|w| (exact fp32).
                # Each core abs-sums its own [IP, O] slice; partials are
                # AllReduced across the 8 cores. Dedicated pool so phase-B
                # prefetches don't WAR-serialize behind absmean slots.
                with tc.tile_pool(name="pha", bufs=2) as pha:
                    for j in range(NA):
                        wa = pha.tile([P, KG * OC], fp32)
                        nc.sync.dma_start(
                            out=wa[:],
                            in_=wp_in[(j // 2) * P:(j // 2 + 1) * P,
                                      (j % 2) * (O // 2):(j % 2 + 1) * (O // 2)])
                        nc.vector.tensor_reduce(
                            partials[:, j:j + 1], wa[:],
                            axis=mybir.AxisListType.X,
                            op=mybir.AluOpType.add,
                            apply_absolute_value=True)
                nc.vector.tensor_reduce(
                    part1[:], partials[:], axis=mybir.AxisListType.X,
                    op=mybir.AluOpType.add)
                nc.gpsimd.dma_start(out=cc_in[:], in_=part1[:])
                if collective:
                    nc.gpsimd.collective_compute(
                        "AllReduce", mybir.AluOpType.add,
                        replica_groups=[list(range(B))],
                        ins=[cc_in[:]], outs=[cc_out[:]])
                else:
                    nc.gpsimd.dma_start(out=cc_out[:], in_=cc_in[:])
                nc.gpsimd.dma_start(out=parts_g[:], in_=cc_out[:])
                nc.vector.memset(ones[:], 1.0)
                nc.vector.memset(ones_row[:], 1.0)
                tsum = psum1.tile([1, 1], fp32)
                nc.tensor.matmul(tsum[:], lhsT=parts_g[:], rhs=ones[:],
                                 start=True, stop=True)
                nc.scalar.activation(tval[:], tsum[:],
                                     mybir.ActivationFunctionType.Copy,
                                     scale=0.7 / float(O * I))
                # broadcast t across partitions on the PE (no DRAM roundtrip)
                tb_ps = psum1.tile([P, 1], fp32, tag="tbps")
                nc.tensor.matmul(tb_ps[:], lhsT=ones_row[:], rhs=tval[:],
                                 start=True, stop=True)
                nc.scalar.activation(t_b[:], tb_ps[:],
                                     mybir.ActivationFunctionType.Copy)
                nc.vector.tensor_scalar_mul(nt_b[:], t_b[:], -1.0)

                # ---- Phase B: per-half quantize + matmul pipeline ----
                preload = {}
                for h in range(NH):
                    if h == 0:
                        # prefetch the first win groups of oc=0 ahead of the
                        # bulky xh loads so the first quantize isn't queued
                        # behind 8MB of x on the same DMA ring
                        for g in range(3):
                            win = win_pool.tile([P, KG, OC], fp32, tag="win")
                            w_src = bass.AP(
                                tensor=wT_in, offset=g * KG * P * O,
                                ap=[[O, P], [P * O, KG], [1, OC]])
                            nc.sync.dma_start(out=win[:], in_=w_src)
                            preload[g] = win
                    # xT half as XQ s-quarter tiles (finer WAR granularity so
                    # h=1 quarters can load under h=0's tail matmuls). For
                    # h=0 only q0 is loaded up front; q1-3 are emitted after
                    # the first quantize chunk to keep the DMA queue clear
                    # for the threshold chain.
                    xq = SH // XQ

                    def _load_xh(q, h=h, xq=xq):
                        xh_t = xh_pool.tile([P, NK, xq], bf16, tag="xh")
                        xh_src = bass.AP(
                            tensor=xT_in, offset=h * SH + q * xq,
                            ap=[[S, P], [P * S, NK], [1, xq]])
                        nc.sync.dma_start(out=xh_t[:], in_=xh_src)
                        return xh_t

                    xhq = [_load_xh(0)]
                    if h > 0:
                        xhq += [_load_xh(q) for q in range(1, XQ)]
                    for oc in range(NOC):
                        # quantize chunk oc into NKG per-k-group sub-tiles
                        # [128, KG, OC] ternary bf16 (fine deps: k=0 matmuls
                        # start as soon as the first group is quantized)
                        wq_g = []
                        for g in range(NKG):
                            k0 = g * KG
                            if h == 0 and oc == 0 and g in preload:
                                win = preload.pop(g)
                            else:
                                win = win_pool.tile([P, KG, OC], fp32,
                                                    tag="win")
                                w_src = bass.AP(
                                    tensor=wT_in,
                                    offset=k0 * P * O + oc * OC,
                                    ap=[[O, P], [P * O, KG], [1, OC]])
                                nc.sync.dma_start(out=win[:], in_=w_src)
                            pt = qtmp.tile([P, KG, OC], bf16, tag="pt")
                            nt = qtmp.tile([P, KG, OC], bf16, tag="nt")
                            wq = wqT_pool.tile([P, KG, OC], bf16)
                            nc.vector.tensor_scalar(
                                pt[:], win[:], t_b[:], None,
                                op0=mybir.AluOpType.is_gt)
                            nc.vector.tensor_scalar(
                                nt[:], win[:], nt_b[:], None,
                                op0=mybir.AluOpType.is_lt)
                            nc.vector.tensor_sub(wq[:], pt[:], nt[:])
                            wq_g.append(wq)
                        if h == 0 and oc == 0:
                            xhq += [_load_xh(q) for q in range(1, XQ)]
                        # matmul: 8 s-blocks x 32 k, N=512
                        for s in range(NSB):
                            sq, so = divmod(s, NSB // XQ)
                            ps = psum_pool.tile([P, OC], fp32)
                            for k in range(NK):
                                nc.tensor.matmul(
                                    ps[:],
                                    lhsT=xhq[sq][:, k, so * P:(so + 1) * P],
                                    rhs=wq_g[k // KG][:, k % KG, :],
                                    start=(k == 0),
                                    stop=(k == NK - 1))
                            ob = outsb.tile([P, OC], bf16)
                            nc.scalar.activation(
                                ob[:], ps[:],
                                mybir.ActivationFunctionType.Copy)
                            nc.scalar.dma_start(
                                out=y_out[
                                    h * SH + s * P:h * SH + (s + 1) * P,
                                    oc * OC:(oc + 1) * OC],
                                in_=ob[:])
                if reps > 1:
                    tc.strict_bb_all_engine_barrier()

    _hoist_waits(nc)
    return nc


_program_cache = {}


def _get_program(reps=1):
    if reps not in _program_cache:
        _program_cache[reps] = build_program(reps=reps)
    return _program_cache[reps]


def make_inputs(x, weight):
    """Host-side marshaling: slice/cast/transpose into per-core input maps."""
    x = np.asarray(x, dtype=np.float32)
    weight = np.asarray(weight, dtype=np.float32)
    assert x.shape == (B, S, I), x.shape
    assert weight.shape == (O, I), weight.shape
    wT = np.ascontiguousarray(weight.T)
    in_maps = []
    for c in range(B):
        xT = np.ascontiguousarray(x[c].T).astype(ml_dtypes.bfloat16)
        wp = np.ascontiguousarray(wT[c * IP:(c + 1) * IP, :])
        in_maps.append({"xT": xT, "wT": wT, "wp": wp})
    return in_maps


def run(x, weight, trace=False, reps=1):
    nc = _get_program(reps=reps)
    in_maps = make_inputs(x, weight)
    res = run_bass_kernel_spmd(nc, in_maps, list(range(B)), trace=trace)
    y = np.stack([res.results[c]["y"] for c in range(B)], axis=0)
    return y.astype(np.float32), res


def kernel(x, weight):
    y, _ = run(x, weight)
    return y


# revision 30
# speedup vs baseline: 79.6337x; 1.0575x over previous
"""BitNet ternary layer on 8 trn2 NeuronCores — v2.

y[b,s,o] = sum_i x[b,s,i] * tq(w)[o,i],  tq(w) = sign(w) * (|w| > 0.7*mean|w|)

Distribution: data-parallel over the batch dim — core c gets x[c] and a
replicated copy of the weight. Host-side marshaling only: x is pre-cast to
bf16 and pre-transposed to xT [I, S]; w is pre-transposed to wT [I, O]
(both are layout/dtype repacks — all of the layer's math stays on device).

Per core:
  A) absmean pass: stream wT fp32 [128, O] i-tiles, DVE abs-reduce to
     per-partition partials, collapse on the PE, scale by 0.7/(O*I) -> t.
  B) for each S-half (SH=1024) and each o-chunk (OC=512):
     quantize: stream wT fp32 [128, KG*OC] tiles, two DVE compares
     (w > t) - (w < -t) -> ternary bf16 directly in matmul-ready
     wqT [i-part, o-free] layout (no transposes anywhere);
     matmul: 8 s-blocks x 32 k-tiles of N=512 bf16 matmuls accumulating
     into PSUM; ACT drains PSUM -> SBUF fp32; DMA stores y [s, o].

Engine budget per core: PE 4096 matmuls ~875us (bottleneck), DMA ~240MB
~670us, DVE ~420us, ACT ~350us. The absmean pass (~180us) serializes at
the start (t gates quantization).
"""
import copy
import sys

sys.path.insert(0, '/opt/trn_rl_repo')

import numpy as np
import ml_dtypes

import concourse.bass as bass
from concourse import mybir
from concourse.tile import TileContext
from concourse.vector_clock import ScopedClock
from concourse.bass_utils import run_bass_kernel_spmd

# ---------------------------------------------------------------------------
# Workarounds for this container's walrus build rejecting sem-waits attached
# to several instruction structs (CTRL/NoOp/Drain/DMA-transpose): emit the
# TileContext exit waits as standalone wait_ge instructions, and post-process
# the module to hoist every immediate sem-wait onto its own single-wait
# InstEventSemaphore (same engine, same program position -> same semantics).
# ---------------------------------------------------------------------------


def _patched_drain_and_barrier(self, tick_clock, wait_clock):
    probe = self.nc.sync.nop()
    wait_clock.add_sem_waits(probe.ins, ScopedClock({None: tick_clock.global_clock}))
    si = probe.ins.sync_info
    waits = list(si.on_wait) if si is not None else []
    if waits:
        probe.ins.sync_info = mybir.SyncInfo(on_wait=[], on_update=list(si.on_update))
        for w in waits:
            self.nc.sync.wait_ge(bass.SemaphoreHandle(w.ant_name, w.id), w.wait_value)
    self.nc.sync.drain()
    self.nc.all_engine_barrier()
    assert self.sems is not None
    popped = self.nc._tile_sem_poison_stack.pop()
    assert popped is self._sem_poison
    self.nc.clear_and_free_semaphores(list(self.sems.allocated().values()))
    self.nc.all_engine_barrier()


TileContext._drain_and_barrier = _patched_drain_and_barrier

_ctr = [0]


def _hoist_waits(nc):
    new_module = copy.replace(nc.m, functions=[])
    for function in nc.m.functions:
        new_function = copy.replace(function, blocks=[])
        new_function.set_allocations_from_list(function.allocations)
        for block in function.blocks:
            new_insts = []
            for inst in block.instructions:
                si = inst.sync_info
                if si is not None and not isinstance(inst, mybir.InstEventSemaphore):
                    imm = [w for w in si.on_wait if w.wait_reg is None]
                    if imm:
                        reg = [w for w in si.on_wait if w.wait_reg is not None]
                        for w in imm:
                            _ctr[0] += 1
                            ev = mybir.InstEventSemaphore(
                                name=f"HW-{_ctr[0]}", ins=[], outs=[])
                            ev.engine = inst.engine
                            ev.sync_info = mybir.SyncInfo(on_wait=[w], on_update=[])
                            new_insts.append(ev)
                        inst.sync_info = mybir.SyncInfo(
                            on_wait=reg, on_update=list(si.on_update))
                new_insts.append(inst)
            new_block = copy.replace(block, instructions=new_insts)
            new_function.blocks.append(new_block)
        new_module.functions.append(new_function)
    nc.m = new_module
    return nc


# ---------------------------------------------------------------------------
# Problem shapes (hardcoded per spec)
# ---------------------------------------------------------------------------
B = 8            # batch -> one per core
S = 2048         # tokens per core
I = 4096         # in features (contraction)
O = 4096         # out features
P = 128
NK = I // P      # 32 k-tiles
OC = 512         # o-chunk width (one PSUM bank at fp32)
NOC = O // OC    # 8
SH = 1024        # token half kept SBUF-resident as xT
NH = S // SH     # 2 halves
NSB = SH // P    # 8 s-tiles per half
KG = 4           # k-tiles per quantize load ([128, KG, OC] fp32 = 1MB DMA)
NKG = NK // KG   # 8 quantize loads per o-chunk
IP = I // B      # 512: per-core slice of i-rows for the sharded absmean
XQ = 4           # xh split into XQ s-quarter tiles for finer WAR deps


def build_program(reps=1, collective=True, nk_mm=NK):
    # nk_mm < NK builds a timing-diagnostic variant (wrong numerics) that
    # only shortens the matmul accumulation depth; all DMA/quantize work is
    # unchanged, so (T(NK) - T(nk_mm)) isolates the pure matmul stream rate.
    fp32 = mybir.dt.float32
    bf16 = mybir.dt.bfloat16
    fp8 = mybir.dt.float8e4

    nc = bass.Bass(num_devices=B if collective else None)
    # xT: x[c] transposed to [I, S], bf16.  wT: weight transposed to [I, O].
    # wp: this core's [IP, O] slice of wT (absmean sharding; host-sliced).
    # y is written bf16 (host casts back to fp32; ~1e-3 extra rel err).
    xT_in = nc.declare_dram_parameter("xT", [I, S], bf16, isOutput=False)
    wT_in = nc.declare_dram_parameter("wT", [I, O], fp32, isOutput=False)
    wp_in = nc.declare_dram_parameter("wp", [IP, O], fp32, isOutput=False)
    y_out = nc.declare_dram_parameter("y", [S, O], bf16, isOutput=True)
    cc_in = nc.dram_tensor("cc_in", [P, 1], fp32)
    cc_out = nc.dram_tensor("cc_out", [P, 1], fp32,
                            addr_space="Shared" if collective else "Local")

    with TileContext(nc) as tc:
        with (
            tc.tile_pool(name="dram", bufs=1, space="DRAM") as dram,
            tc.tile_pool(name="singles", bufs=1) as singles,
            tc.tile_pool(name="psum1", bufs=1, space="PSUM") as psum1,
            tc.tile_pool(name="psum", bufs=7, space="PSUM") as psum_pool,
            tc.tile_pool(name="outsb", bufs=4) as outsb,
            tc.tile_pool(name="xh_pool", bufs=4) as xh_pool,
            tc.tile_pool(name="wqT_pool", bufs=2 * NKG) as wqT_pool,
            tc.tile_pool(name="win_pool", bufs=3) as win_pool,
            tc.tile_pool(name="qtmp", bufs=2) as qtmp,
        ):
            # fp8 cache of the ternary weights, quantized once during h=0
            # ({-1,0,+1} are exact in fp8e4; 16MB vs re-reading 64MB fp32)
            wq8 = [dram.tile([P, NK, OC], fp8, name=f"wq8_{oc}")
                   for oc in range(NOC)]
            NA = (IP // P) * 2  # absmean tiles: [P, O//2] fp32 over wp
            partials = singles.tile([P, NA], fp32)
            part1 = singles.tile([P, 1], fp32)
            parts_g = singles.tile([P, 1], fp32)
            ones = singles.tile([P, 1], fp32)
            ones_row = singles.tile([1, P], fp32)
            tval = singles.tile([1, 1], fp32)
            t_b = singles.tile([P, 1], fp32)
            nt_b = singles.tile([P, 1], fp32)

            for rep in range(reps):
                # ---- Phase A: threshold t = 0.7 * mean|w| (exact fp32).
                # Each core abs-sums its own [IP, O] slice; partials are
                # AllReduced across the 8 cores. Dedicated pool so phase-B
                # prefetches don't WAR-serialize behind absmean slots.
                with tc.tile_pool(name="pha", bufs=2) as pha:
                    for j in range(NA):
                        wa = pha.tile([P, KG * OC], fp32)
                        nc.sync.dma_start(
                            out=wa[:],
                            in_=wp_in[(j // 2) * P:(j // 2 + 1) * P,
                                      (j % 2) * (O // 2):(j % 2 + 1) * (O // 2)])
                        nc.vector.tensor_reduce(
                            partials[:, j:j + 1], wa[:],
                            axis=mybir.AxisListType.X,
                            op=mybir.AluOpType.add,
                            apply_absolute_value=True)
                nc.vector.tensor_reduce(
                    part1[:], partials[:], axis=mybir.AxisListType.X,
                    op=mybir.AluOpType.add)
                nc.gpsimd.dma_start(out=cc_in[:], in_=part1[:])
                if collective:
                    nc.gpsimd.collective_compute(
                        "AllReduce", mybir.AluOpType.add,
                        replica_groups=[list(range(B))],
                        ins=[cc_in[:]], outs=[cc_out[:]])
                else:
                    nc.gpsimd.dma_start(out=cc_out[:], in_=cc_in[:])
                nc.gpsimd.dma_start(out=parts_g[:], in_=cc_out[:])
                nc.vector.memset(ones[:], 1.0)
                nc.vector.memset(ones_row[:], 1.0)
                tsum = psum1.tile([1, 1], fp32)
                nc.tensor.matmul(tsum[:], lhsT=parts_g[:], rhs=ones[:],
                                 start=True, stop=True)
                nc.scalar.activation(tval[:], tsum[:],
                                     mybir.ActivationFunctionType.Copy,
                                     scale=0.7 / float(O * I))
                # broadcast t across partitions on the PE (no DRAM roundtrip)
                tb_ps = psum1.tile([P, 1], fp32, tag="tsum")
                nc.tensor.matmul(tb_ps[:], lhsT=ones_row[:], rhs=tval[:],
                                 start=True, stop=True)
                nc.scalar.activation(t_b[:], tb_ps[:],
                                     mybir.ActivationFunctionType.Copy)
                nc.vector.tensor_scalar_mul(nt_b[:], t_b[:], -1.0)

                # ---- Phase B: per-half quantize + matmul pipeline ----
                preload = {}
                for h in range(NH):
                    if h == 0:
                        # prefetch the first win groups of oc=0 ahead of the
                        # bulky xh loads so the first quantize isn't queued
                        # behind 8MB of x on the same DMA ring
                        for g in range(3):
                            win = win_pool.tile([P, KG, OC], fp32, tag="win")
                            w_src = bass.AP(
                                tensor=wT_in, offset=g * KG * P * O,
                                ap=[[O, P], [P * O, KG], [1, OC]])
                            nc.sync.dma_start(out=win[:], in_=w_src)
                            preload[g] = win
                    # xT half as XQ s-quarter tiles (finer WAR granularity so
                    # h=1 quarters can load under h=0's tail matmuls). For
                    # h=0 only q0 is loaded up front; q1-3 are emitted after
                    # the first quantize chunk to keep the DMA queue clear
                    # for the threshold chain.
                    xq = SH // XQ

                    def _load_xh(q, h=h, xq=xq):
                        xh_t = xh_pool.tile([P, NK, xq], bf16, tag="xh")
                        xh_src = bass.AP(
                            tensor=xT_in, offset=h * SH + q * xq,
                            ap=[[S, P], [P * S, NK], [1, xq]])
                        nc.sync.dma_start(out=xh_t[:], in_=xh_src)
                        return xh_t

                    xhq = [_load_xh(0)]
                    if h > 0:
                        xhq += [_load_xh(q) for q in range(1, XQ)]
                    for oc in range(NOC):
                        # quantize chunk oc into NKG per-k-group sub-tiles
                        # [128, KG, OC] ternary bf16 (fine deps: k=0 matmuls
                        # start as soon as the first group is quantized)
                        wq_g = []
                        for g in range(NKG):
                            k0 = g * KG
                            wq = wqT_pool.tile([P, KG, OC], bf16)
                            if h == 0:
                                if oc == 0 and g in preload:
                                    win = preload.pop(g)
                                else:
                                    win = win_pool.tile([P, KG, OC], fp32,
                                                        tag="win")
                                    w_src = bass.AP(
                                        tensor=wT_in,
                                        offset=k0 * P * O + oc * OC,
                                        ap=[[O, P], [P * O, KG], [1, OC]])
                                    nc.sync.dma_start(out=win[:], in_=w_src)
                                pt = qtmp.tile([P, KG, OC], bf16, tag="pt")
                                nt = qtmp.tile([P, KG, OC], bf16, tag="nt")
                                nc.vector.tensor_scalar(
                                    pt[:], win[:], t_b[:], None,
                                    op0=mybir.AluOpType.is_gt)
                                nc.vector.tensor_scalar(
                                    nt[:], win[:], nt_b[:], None,
                                    op0=mybir.AluOpType.is_lt)
                                nc.vector.tensor_sub(wq[:], pt[:], nt[:])
                                q8 = qtmp.tile([P, KG, OC], fp8, tag="q8")
                                nc.scalar.activation(
                                    q8[:], wq[:],
                                    mybir.ActivationFunctionType.Copy)
                                nc.scalar.dma_start(
                                    out=wq8[oc][:, k0:k0 + KG, :], in_=q8[:])
                            else:
                                l8 = win_pool.tile([P, KG, OC], fp8,
                                                   tag="l8")
                                nc.sync.dma_start(
                                    out=l8[:], in_=wq8[oc][:, k0:k0 + KG, :])
                                nc.vector.tensor_copy(wq[:], l8[:])
                            wq_g.append(wq)
                        if h == 0 and oc == 0:
                            xhq += [_load_xh(q) for q in range(1, XQ)]
                        # matmul: 8 s-blocks x 32 k, N=512
                        for s in range(NSB):
                            sq, so = divmod(s, NSB // XQ)
                            ps = psum_pool.tile([P, OC], fp32)
                            for k in range(nk_mm):
                                nc.tensor.matmul(
                                    ps[:],
                                    lhsT=xhq[sq][:, k, so * P:(so + 1) * P],
                                    rhs=wq_g[k // KG][:, k % KG, :],
                                    start=(k == 0),
                                    stop=(k == nk_mm - 1))
                            ob = outsb.tile([P, OC], bf16)
                            nc.scalar.activation(
                                ob[:], ps[:],
                                mybir.ActivationFunctionType.Copy)
                            nc.scalar.dma_start(
                                out=y_out[
                                    h * SH + s * P:h * SH + (s + 1) * P,
                                    oc * OC:(oc + 1) * OC],
                                in_=ob[:])
                if reps > 1:
                    tc.strict_bb_all_engine_barrier()

    _hoist_waits(nc)
    return nc


_program_cache = {}


def _get_program(reps=1):
    if reps not in _program_cache:
        _program_cache[reps] = build_program(reps=reps)
    return _program_cache[reps]


def make_inputs(x, weight):
    """Host-side marshaling: slice/cast/transpose into per-core input maps."""
    x = np.asarray(x, dtype=np.float32)
    weight = np.asarray(weight, dtype=np.float32)
    assert x.shape == (B, S, I), x.shape
    assert weight.shape == (O, I), weight.shape
    wT = np.ascontiguousarray(weight.T)
    in_maps = []
    for c in range(B):
        xT = np.ascontiguousarray(x[c].T).astype(ml_dtypes.bfloat16)
        wp = np.ascontiguousarray(wT[c * IP:(c + 1) * IP, :])
        in_maps.append({"xT": xT, "wT": wT, "wp": wp})
    return in_maps


def run(x, weight, trace=False, reps=1):
    nc = _get_program(reps=reps)
    in_maps = make_inputs(x, weight)
    res = run_bass_kernel_spmd(nc, in_maps, list(range(B)), trace=trace)
    y = np.stack([res.results[c]["y"] for c in range(B)], axis=0)
    return y.astype(np.float32), res


def kernel(x, weight):
    y, _ = run(x, weight)
    return y


# revision 37
# speedup vs baseline: 80.3447x; 1.0089x over previous
"""BitNet ternary layer on 8 trn2 NeuronCores.

y[b,s,o] = sum_i x[b,s,i] * tq(w)[o,i],  tq(w) = sign(w) * (|w| > 0.7*mean|w|)

Distribution: data-parallel over the batch dim — core c computes x[c] @ tq(w).T
with a replicated weight. Host side does marshaling only (layout/dtype
repacks; all of the layer's math runs on device): x[c] is pre-cast to bf16
and pre-transposed to xT [I, S]; w is pre-transposed to wT [I, O] and
pre-sliced per core for the sharded absmean.

Per core:
  A) threshold: each core abs-sums its own 1/8 slice of wT (fp32, exact),
     the [128,1] partials are AllReduced across the 8 cores, collapsed on
     the PE, scaled by 0.7/(O*I) -> t, and PE-broadcast across partitions.
  B) h=0 half (tokens 0-1023): stream wT fp32 tiles, two DVE compares
     (w > t) - (w < -t) -> ternary bf16 directly in matmul-ready
     wqT [i-part, o-free] layout (no transposes anywhere on device);
     ACT also packs each quantized tile to fp8e4 ({-1,0,+1} exact) and
     caches it in DRAM (16MB instead of re-reading 64MB of fp32).
     matmul: per o-chunk (OC=512 = one PSUM bank), 8 s-blocks x 32 k-tile
     N=512 bf16 matmuls accumulate in PSUM; ACT drains PSUM -> bf16; DMA
     stores y (host casts back to fp32).
  C) h=1 half: reload the fp8 cache, DVE-expand to bf16, same matmuls.

Engine budget per core: PE 4096 matmuls x ~215ns = ~880us (the bottleneck,
>92% busy); DMA ~140MB; DVE/ACT well under. Startup (absmean+AllReduce+
first quantize chunk) ~60-90us. Measured ~1.08ms/rep on HW.
"""
import copy
import sys

sys.path.insert(0, '/opt/trn_rl_repo')

import numpy as np
import ml_dtypes

import concourse.bass as bass
from concourse import mybir
from concourse.tile import TileContext
from concourse.vector_clock import ScopedClock
from concourse.bass_utils import run_bass_kernel_spmd

# ---------------------------------------------------------------------------
# Workarounds for this container's walrus build rejecting sem-waits attached
# to several instruction structs (CTRL/NoOp/Drain/DMA-transpose): emit the
# TileContext exit waits as standalone wait_ge instructions, and post-process
# the module to hoist every immediate sem-wait onto its own single-wait
# InstEventSemaphore (same engine, same program position -> same semantics).
# ---------------------------------------------------------------------------


def _patched_drain_and_barrier(self, tick_clock, wait_clock):
    probe = self.nc.sync.nop()
    wait_clock.add_sem_waits(probe.ins, ScopedClock({None: tick_clock.global_clock}))
    si = probe.ins.sync_info
    waits = list(si.on_wait) if si is not None else []
    if waits:
        probe.ins.sync_info = mybir.SyncInfo(on_wait=[], on_update=list(si.on_update))
        for w in waits:
            self.nc.sync.wait_ge(bass.SemaphoreHandle(w.ant_name, w.id), w.wait_value)
    self.nc.sync.drain()
    self.nc.all_engine_barrier()
    assert self.sems is not None
    popped = self.nc._tile_sem_poison_stack.pop()
    assert popped is self._sem_poison
    self.nc.clear_and_free_semaphores(list(self.sems.allocated().values()))
    self.nc.all_engine_barrier()


TileContext._drain_and_barrier = _patched_drain_and_barrier

_ctr = [0]


def _hoist_waits(nc):
    new_module = copy.replace(nc.m, functions=[])
    for function in nc.m.functions:
        new_function = copy.replace(function, blocks=[])
        new_function.set_allocations_from_list(function.allocations)
        for block in function.blocks:
            new_insts = []
            for inst in block.instructions:
                si = inst.sync_info
                if si is not None and not isinstance(inst, mybir.InstEventSemaphore):
                    imm = [w for w in si.on_wait if w.wait_reg is None]
                    if imm:
                        reg = [w for w in si.on_wait if w.wait_reg is not None]
                        for w in imm:
                            _ctr[0] += 1
                            ev = mybir.InstEventSemaphore(
                                name=f"HW-{_ctr[0]}", ins=[], outs=[])
                            ev.engine = inst.engine
                            ev.sync_info = mybir.SyncInfo(on_wait=[w], on_update=[])
                            new_insts.append(ev)
                        inst.sync_info = mybir.SyncInfo(
                            on_wait=reg, on_update=list(si.on_update))
                new_insts.append(inst)
            new_block = copy.replace(block, instructions=new_insts)
            new_function.blocks.append(new_block)
        new_module.functions.append(new_function)
    nc.m = new_module
    return nc


# ---------------------------------------------------------------------------
# Problem shapes (hardcoded per spec)
# ---------------------------------------------------------------------------
B = 8            # batch -> one per core
S = 2048         # tokens per core
I = 4096         # in features (contraction)
O = 4096         # out features
P = 128
NK = I // P      # 32 k-tiles
OC = 512         # o-chunk width (one PSUM bank at fp32)
NOC = O // OC    # 8
SH = 1024        # token half kept SBUF-resident as xT
NH = S // SH     # 2 halves
NSB = SH // P    # 8 s-tiles per half
KG = 4           # k-tiles per quantize load ([128, KG, OC] fp32 = 1MB DMA)
NKG = NK // KG   # 8 quantize loads per o-chunk
IP = I // B      # 512: per-core slice of i-rows for the sharded absmean
XQ = 2           # xh split into XQ s-quarter tiles for finer WAR deps


def build_program(reps=1, collective=True, nk_mm=NK):
    # nk_mm < NK builds a timing-diagnostic variant (wrong numerics) that
    # only shortens the matmul accumulation depth; all DMA/quantize work is
    # unchanged, so (T(NK) - T(nk_mm)) isolates the pure matmul stream rate.
    fp32 = mybir.dt.float32
    bf16 = mybir.dt.bfloat16
    fp8 = mybir.dt.float8e4

    nc = bass.Bass(num_devices=B if collective else None)
    # xT: x[c] transposed to [I, S], bf16.  wT: weight transposed to [I, O].
    # wp: this core's [IP, O] slice of wT (absmean sharding; host-sliced).
    # y is written bf16 (host casts back to fp32; ~1e-3 extra rel err).
    xT_in = nc.declare_dram_parameter("xT", [I, S], bf16, isOutput=False)
    wT_in = nc.declare_dram_parameter("wT", [I, O], fp32, isOutput=False)
    wp_in = nc.declare_dram_parameter("wp", [IP, O], fp32, isOutput=False)
    y_out = nc.declare_dram_parameter("y", [S, O], bf16, isOutput=True)
    cc_in = nc.dram_tensor("cc_in", [P, 1], fp32)
    cc_out = nc.dram_tensor("cc_out", [P, 1], fp32,
                            addr_space="Shared" if collective else "Local")

    with TileContext(nc) as tc:
        with (
            tc.tile_pool(name="dram", bufs=1, space="DRAM") as dram,
            tc.tile_pool(name="singles", bufs=1) as singles,
            tc.tile_pool(name="psum1", bufs=1, space="PSUM") as psum1,
            tc.tile_pool(name="psum", bufs=7, space="PSUM") as psum_pool,
            tc.tile_pool(name="outsb", bufs=6) as outsb,
            tc.tile_pool(name="xh_pool", bufs=4) as xh_pool,
            tc.tile_pool(name="wqT_pool", bufs=2 * NKG) as wqT_pool,
            tc.tile_pool(name="win_pool", bufs=3) as win_pool,
            tc.tile_pool(name="qtmp", bufs=2) as qtmp,
        ):
            # fp8 cache of the ternary weights, quantized once during h=0
            # ({-1,0,+1} are exact in fp8e4; 16MB vs re-reading 64MB fp32)
            wq8 = [dram.tile([P, NK, OC], fp8, name=f"wq8_{oc}")
                   for oc in range(NOC)]
            NA = (IP // P) * 2  # absmean tiles: [P, O//2] fp32 over wp
            partials = singles.tile([P, NA], fp32)
            part1 = singles.tile([P, 1], fp32)
            parts_g = singles.tile([P, 1], fp32)
            ones = singles.tile([P, 1], fp32)
            ones_row = singles.tile([1, P], fp32)
            tval = singles.tile([1, 1], fp32)
            t_b = singles.tile([P, 1], fp32)
            nt_b = singles.tile([P, 1], fp32)

            for rep in range(reps):
                # ---- Phase A: threshold t = 0.7 * mean|w| (exact fp32).
                # Each core abs-sums its own [IP, O] slice; partials are
                # AllReduced across the 8 cores. Dedicated pool so phase-B
                # prefetches don't WAR-serialize behind absmean slots.
                with tc.tile_pool(name="pha", bufs=2) as pha:
                    for j in range(NA):
                        wa = pha.tile([P, O // 2], fp32)
                        nc.sync.dma_start(
                            out=wa[:],
                            in_=wp_in[(j // 2) * P:(j // 2 + 1) * P,
                                      (j % 2) * (O // 2):(j % 2 + 1) * (O // 2)])
                        nc.vector.tensor_reduce(
                            partials[:, j:j + 1], wa[:],
                            axis=mybir.AxisListType.X,
                            op=mybir.AluOpType.add,
                            apply_absolute_value=True)
                nc.vector.tensor_reduce(
                    part1[:], partials[:], axis=mybir.AxisListType.X,
                    op=mybir.AluOpType.add)
                nc.gpsimd.dma_start(out=cc_in[:], in_=part1[:])
                if collective:
                    nc.gpsimd.collective_compute(
                        "AllReduce", mybir.AluOpType.add,
                        replica_groups=[list(range(B))],
                        ins=[cc_in[:]], outs=[cc_out[:]])
                else:
                    nc.gpsimd.dma_start(out=cc_out[:], in_=cc_in[:])
                nc.gpsimd.dma_start(out=parts_g[:], in_=cc_out[:])
                nc.vector.memset(ones[:], 1.0)
                nc.vector.memset(ones_row[:], 1.0)
                tsum = psum1.tile([1, 1], fp32)
                nc.tensor.matmul(tsum[:], lhsT=parts_g[:], rhs=ones[:],
                                 start=True, stop=True)
                nc.scalar.activation(tval[:], tsum[:],
                                     mybir.ActivationFunctionType.Copy,
                                     scale=0.7 / float(O * I))
                # broadcast t across partitions on the PE (no DRAM roundtrip)
                tb_ps = psum1.tile([P, 1], fp32, tag="tsum")
                nc.tensor.matmul(tb_ps[:], lhsT=ones_row[:], rhs=tval[:],
                                 start=True, stop=True)
                nc.scalar.activation(t_b[:], tb_ps[:],
                                     mybir.ActivationFunctionType.Copy)
                nc.vector.tensor_scalar_mul(nt_b[:], t_b[:], -1.0)

                # ---- Phase B: per-half quantize + matmul pipeline ----
                preload = {}
                for h in range(NH):
                    if h == 0:
                        # prefetch the first win groups of oc=0 ahead of the
                        # bulky xh loads so the first quantize isn't queued
                        # behind 8MB of x on the same DMA ring
                        for g in range(3):
                            win = win_pool.tile([P, KG, OC], fp32, tag="win")
                            w_src = bass.AP(
                                tensor=wT_in, offset=g * KG * P * O,
                                ap=[[O, P], [P * O, KG], [1, OC]])
                            nc.sync.dma_start(out=win[:], in_=w_src)
                            preload[g] = win
                    # xT half as XQ s-quarter tiles (finer WAR granularity so
                    # h=1 quarters can load under h=0's tail matmuls). For
                    # h=0 only q0 is loaded up front; q1-3 are emitted after
                    # the first quantize chunk to keep the DMA queue clear
                    # for the threshold chain.
                    xq = SH // XQ

                    def _load_xh(q, h=h, xq=xq):
                        xh_t = xh_pool.tile([P, NK, xq], bf16, tag="xh")
                        xh_src = bass.AP(
                            tensor=xT_in, offset=h * SH + q * xq,
                            ap=[[S, P], [P * S, NK], [1, xq]])
                        nc.sync.dma_start(out=xh_t[:], in_=xh_src)
                        return xh_t

                    xhq = [_load_xh(0)]
                    if h > 0:
                        xhq += [_load_xh(q) for q in range(1, XQ)]
                    for oc in range(NOC):
                        # quantize chunk oc into NKG per-k-group sub-tiles
                        # [128, KG, OC] ternary bf16 (fine deps: k=0 matmuls
                        # start as soon as the first group is quantized)
                        wq_g = []
                        for g in range(NKG):
                            k0 = g * KG
                            wq = wqT_pool.tile([P, KG, OC], bf16)
                            if h == 0:
                                if oc == 0 and g in preload:
                                    win = preload.pop(g)
                                else:
                                    win = win_pool.tile([P, KG, OC], fp32,
                                                        tag="win")
                                    w_src = bass.AP(
                                        tensor=wT_in,
                                        offset=k0 * P * O + oc * OC,
                                        ap=[[O, P], [P * O, KG], [1, OC]])
                                    nc.sync.dma_start(out=win[:], in_=w_src)
                                pt = qtmp.tile([P, KG, OC], bf16, tag="pt")
                                nt = qtmp.tile([P, KG, OC], bf16, tag="nt")
                                nc.vector.tensor_scalar(
                                    pt[:], win[:], t_b[:], None,
                                    op0=mybir.AluOpType.is_gt)
                                nc.vector.tensor_scalar(
                                    nt[:], win[:], nt_b[:], None,
                                    op0=mybir.AluOpType.is_lt)
                                nc.vector.tensor_sub(wq[:], pt[:], nt[:])
                                q8 = qtmp.tile([P, KG, OC], fp8, tag="q8")
                                nc.scalar.activation(
                                    q8[:], wq[:],
                                    mybir.ActivationFunctionType.Copy)
                                nc.scalar.dma_start(
                                    out=wq8[oc][:, k0:k0 + KG, :], in_=q8[:])
                            else:
                                l8 = win_pool.tile([P, KG, OC], fp8,
                                                   tag="l8")
                                nc.sync.dma_start(
                                    out=l8[:], in_=wq8[oc][:, k0:k0 + KG, :])
                                nc.vector.tensor_copy(wq[:], l8[:])
                            wq_g.append(wq)
                        if h == 0 and oc == 0:
                            xhq += [_load_xh(q) for q in range(1, XQ)]
                        # matmul: 8 s-blocks x 32 k, N=512
                        for s in range(NSB):
                            sq, so = divmod(s, NSB // XQ)
                            ps = psum_pool.tile([P, OC], fp32)
                            for k in range(nk_mm):
                                nc.tensor.matmul(
                                    ps[:],
                                    lhsT=xhq[sq][:, k, so * P:(so + 1) * P],
                                    rhs=wq_g[k // KG][:, k % KG, :],
                                    start=(k == 0),
                                    stop=(k == nk_mm - 1))
                            ob = outsb.tile([P, OC], bf16)
                            nc.scalar.activation(
                                ob[:], ps[:],
                                mybir.ActivationFunctionType.Copy)
                            nc.scalar.dma_start(
                                out=y_out[
                                    h * SH + s * P:h * SH + (s + 1) * P,
                                    oc * OC:(oc + 1) * OC],
                                in_=ob[:])
                if reps > 1:
                    tc.strict_bb_all_engine_barrier()

    _hoist_waits(nc)
    return nc


_program_cache = {}


def _get_program(reps=1):
    if reps not in _program_cache:
        _program_cache[reps] = build_program(reps=reps)
    return _program_cache[reps]


def make_inputs(x, weight):
    """Host-side marshaling: slice/cast/transpose into per-core input maps."""
    x = np.asarray(x, dtype=np.float32)
    weight = np.asarray(weight, dtype=np.float32)
    assert x.shape == (B, S, I), x.shape
    assert weight.shape == (O, I), weight.shape
    wT = np.ascontiguousarray(weight.T)
    in_maps = []
    for c in range(B):
        xT = np.ascontiguousarray(x[c].T).astype(ml_dtypes.bfloat16)
        wp = np.ascontiguousarray(wT[c * IP:(c + 1) * IP, :])
        in_maps.append({"xT": xT, "wT": wT, "wp": wp})
    return in_maps


def run(x, weight, trace=False, reps=1):
    nc = _get_program(reps=reps)
    in_maps = make_inputs(x, weight)
    res = run_bass_kernel_spmd(nc, in_maps, list(range(B)), trace=trace)
    y = np.stack([res.results[c]["y"] for c in range(B)], axis=0)
    return y.astype(np.float32), res


def kernel(x, weight):
    y, _ = run(x, weight)
    return y
